# revision 4
# baseline (speedup 1.0000x reference)
"""MoE routing gate kernel for Trainium2 (Bass/Tile), 8 NeuronCores.

reference:
    scores = sigmoid(x @ E^T) + bias          # (N, 64)
    top_values, indices = top_k(scores, 8)    # (N, 8)
    weights = top_values / sum(top_values)
    return weights(f32), indices(int32), scores(f32)

Strategy (data-parallel over tokens, 8 cores, 16384 tokens each):
  - x shard is fed to the device transposed (hidden-major) so the PE
    contraction dim (hidden) lands on SBUF partitions with plain
    full-rate DMAs.  E^T (2048x64) and a broadcast bias are replicated.
  - Per 512-token tile: one 4MB DMA in -> 16 accumulating matmuls
    (K=128 chunks) into PSUM scoresT[64,512] -> sigmoid on eviction
    (scalar engine) -> 4 PE transposes back to [128,64] token-major ->
    bias add -> native vector-engine max8/max_index top-k -> normalize
    -> pack and DMA out.
"""

import os
import sys
import threading
from contextlib import ExitStack

import numpy as np

N_TOKENS = 131072
HIDDEN = 2048
EXPERTS = 64
TOPK = 8
N_CORES = 8
TOK_PER_CORE = N_TOKENS // N_CORES  # 16384

TILE_TOK = 512                      # tokens per tile
N_TILES = TOK_PER_CORE // TILE_TOK  # 32
KC = HIDDEN // 128                  # 16 contraction chunks of 128
GROUPS = TILE_TOK // 128            # 4 128-token groups per tile

# fp32r: fp32 bits, full-rate PE streaming mode (vs 4x slower plain fp32).
USE_F32R = os.environ.get("GATE_F32R", "0") == "1"

_prog_cache = {}


def _ensure_path():
    for p in ("/opt/trn_rl_repo",):
        if p not in sys.path and os.path.isdir(p):
            sys.path.insert(0, p)


def _build_program():
    _ensure_path()
    import concourse.bass as bass
    import concourse.mybir as mybir
    import concourse.tile as tile
    from concourse import bacc
    from concourse.bass import ts
    from concourse.masks import make_identity

    f32 = mybir.dt.float32
    f32r = mybir.dt.float32r
    u32 = mybir.dt.uint32

    nc = bacc.Bacc(
        "TRN2", target_bir_lowering=False, debug=False, num_devices=N_CORES
    )

    xt = nc.dram_tensor(
        "xt", [HIDDEN, TOK_PER_CORE], f32, kind="ExternalInput"
    ).ap()
    et = nc.dram_tensor("et", [HIDDEN, EXPERTS], f32, kind="ExternalInput").ap()
    biasb = nc.dram_tensor(
        "biasb", [128, GROUPS * EXPERTS], f32, kind="ExternalInput"
    ).ap()
    scores_o = nc.dram_tensor(
        "scores", [TOK_PER_CORE, EXPERTS], f32, kind="ExternalOutput"
    ).ap()
    w_o = nc.dram_tensor("w", [TOK_PER_CORE, TOPK], f32, kind="ExternalOutput").ap()
    idx_o = nc.dram_tensor(
        "idx", [TOK_PER_CORE, TOPK], u32, kind="ExternalOutput"
    ).ap()

    # DRAM views:
    # xt rows h = c*128 + p  ->  [p, c, tok]
    xt_v = xt.rearrange("(c p) n -> p c n", p=128)
    et_v = et.rearrange("(c p) e -> p c e", p=128)
    # token t within tile = g*128 + p  ->  [p, g, ...]
    sco_v = scores_o.rearrange("(i g p) e -> i p g e", p=128, g=GROUPS)
    w_v = w_o.rearrange("(i g p) k -> i p g k", p=128, g=GROUPS)
    idx_v = idx_o.rearrange("(i g p) k -> i p g k", p=128, g=GROUPS)

    with tile.TileContext(nc) as tc, ExitStack() as ctx:
        const_pool = ctx.enter_context(tc.tile_pool(name="const", bufs=1))
        xpool = ctx.enter_context(tc.tile_pool(name="x", bufs=3))
        pspool = ctx.enter_context(
            tc.tile_pool(name="ps", bufs=2, space="PSUM")
        )
        stpool = ctx.enter_context(tc.tile_pool(name="st", bufs=2))
        ptpool = ctx.enter_context(
            tc.tile_pool(name="pt", bufs=2, space="PSUM")
        )
        scpool = ctx.enter_context(tc.tile_pool(name="sc", bufs=3))
        smalls = ctx.enter_context(tc.tile_pool(name="smalls", bufs=4))
        outpool = ctx.enter_context(tc.tile_pool(name="outs", bufs=3))

        ident = const_pool.tile([128, 128], f32)
        make_identity(nc, ident[:])

        et_sb = const_pool.tile([128, KC, EXPERTS], f32)
        nc.sync.dma_start(out=et_sb[:], in_=et_v)
        bias_sb = const_pool.tile([128, GROUPS * EXPERTS], f32)
        nc.sync.dma_start(out=bias_sb[:], in_=biasb)

        mm_dt = f32r if USE_F32R else f32

        for i in range(N_TILES):
            x_sb = xpool.tile([128, KC, TILE_TOK], f32)
            nc.sync.dma_start(out=x_sb[:], in_=xt_v[:, :, ts(i, TILE_TOK)])

            ps = pspool.tile([EXPERTS, TILE_TOK], f32)
            for c in range(KC):
                nc.tensor.matmul(
                    ps[:],
                    lhsT=et_sb[:, c, :].bitcast(mm_dt),
                    rhs=x_sb[:, c, :].bitcast(mm_dt),
                    start=(c == 0),
                    stop=(c == KC - 1),
                )

            # e = exp(-dot); scores = 1/(1+e) + bias.  This 3-op expansion
            # (ACT Exp table, fp32 add, DVE reciprocal) matches the XLA-neuron
            # lowering of jax.nn.sigmoid bitwise, which keeps top-k tie sets
            # (scores exactly 1.0) identical to the reference.
            st = stpool.tile([EXPERTS, TILE_TOK], f32)
            nc.scalar.activation(
                st[:], ps[:], mybir.ActivationFunctionType.Exp, scale=-1.0
            )

            pt = ptpool.tile([128, GROUPS * EXPERTS], f32)
            for g in range(GROUPS):
                nc.tensor.transpose(
                    pt[:, ts(g, EXPERTS)],
                    st[:, ts(g, 128)],
                    ident[0:EXPERTS, 0:EXPERTS],
                )

            den = scpool.tile([128, GROUPS * EXPERTS], f32, tag="den")
            nc.vector.tensor_scalar_add(den[:], pt[:], 1.0)
            rcp = scpool.tile([128, GROUPS * EXPERTS], f32, tag="rcp")
            nc.vector.reciprocal(rcp[:], den[:])
            sc = scpool.tile([128, GROUPS * EXPERTS], f32, tag="sc")
            nc.vector.tensor_add(sc[:], rcp[:], bias_sb[:])

            wpack = outpool.tile([128, GROUPS, TOPK], f32, tag="wpack")
            ipack = outpool.tile([128, GROUPS, TOPK], u32, tag="ipack")
            for g in range(GROUPS):
                tv = smalls.tile([128, TOPK], f32, tag="tv")
                nc.vector.max(tv[:], sc[:, ts(g, EXPERTS)])
                nc.vector.max_index(ipack[:, g, :], tv[:], sc[:, ts(g, EXPERTS)])
                ssum = smalls.tile([128, 1], f32, tag="ssum")
                nc.vector.tensor_reduce(
                    ssum[:], tv[:], axis=mybir.AxisListType.X, op=mybir.AluOpType.add
                )
                rec = smalls.tile([128, 1], f32, tag="rec")
                nc.vector.reciprocal(rec[:], ssum[:])
                nc.vector.tensor_scalar_mul(wpack[:, g, :], tv[:], rec[:])

            nc.sync.dma_start(
                out=sco_v[i],
                in_=sc[:].rearrange("p (g e) -> p g e", g=GROUPS),
            )
            nc.sync.dma_start(out=w_v[i], in_=wpack[:])
            nc.sync.dma_start(out=idx_v[i], in_=ipack[:])

    nc.compile()
    return nc


def _get_program():
    if "nc" not in _prog_cache:
        _prog_cache["nc"] = _build_program()
    return _prog_cache["nc"]


def _prep_inputs(x, expert_embeddings, bias):
    x = np.ascontiguousarray(x, dtype=np.float32)
    et = np.ascontiguousarray(expert_embeddings.T, dtype=np.float32)  # (2048, 64)
    biasb = np.tile(np.asarray(bias, dtype=np.float32), (128, GROUPS))

    shards = [None] * N_CORES

    def _one(c):
        xs = x[c * TOK_PER_CORE : (c + 1) * TOK_PER_CORE]
        shards[c] = np.ascontiguousarray(xs.T)  # (2048, 16384)

    threads = [threading.Thread(target=_one, args=(c,)) for c in range(N_CORES)]
    for t in threads:
        t.start()
    for t in threads:
        t.join()

    return [{"xt": shards[c], "et": et, "biasb": biasb} for c in range(N_CORES)]


def _run(x, expert_embeddings, bias, trace=False, **trace_kwargs):
    _ensure_path()
    from concourse.bass_utils import run_bass_kernel_spmd

    nc = _get_program()
    in_maps = _prep_inputs(x, expert_embeddings, bias)
    res = run_bass_kernel_spmd(
        nc, in_maps, core_ids=list(range(N_CORES)), trace=trace, **trace_kwargs
    )
    weights = np.concatenate([r["w"] for r in res.results], axis=0)
    indices = np.concatenate(
        [r["idx"] for r in res.results], axis=0
    ).view(np.int32)
    scores = np.concatenate([r["scores"] for r in res.results], axis=0)
    return (weights, indices, scores), res


def kernel(x, expert_embeddings, bias):
    out, _ = _run(x, expert_embeddings, bias, trace=False)
    return out
